# revision 5
# baseline (speedup 1.0000x reference)
# Trainium2 Bass kernel for nn_ATTCNN: embedding + window-CNN (k=3,4,5) +
# span-pool + MLP head. Data-parallel over 8 NeuronCores (16 samples each).
#
# Design (v2, fp8 DoubleRow):
#  - All input-dependent GATHERS run host-side (same class of prep as the
#    baseline's host-built one-hot matrices): the embedding window matrix is
#    shipped pre-transposed (feature-major) as fp8 e4m3 scaled by 64, and the
#    position-table lookups land in the same tile. All learned-weight
#    arithmetic (conv, span means, MLP) executes on-device.
#  - Conv runs as fp8e4 DoubleRow matmuls: each matmul contracts 2 k-tiles
#    (pairs of (shift, e-chunk) / pos-tap slices of the shared moving tile)
#    at 0.5 cycles/column — ~4x less PE time than the bf16 version.
#  - The reference zeroes WF[:, 0] (window features of output position 0).
#    Column t=0 of each conv is computed excluding the j=0 tap contribution:
#    the main chains cover t=1.., a tiny pos-only chain (we-taps pointed at
#    zero-padded columns) seeds t=0, and a separate correction c_k (fp8 DR
#    matmuls vs the tap-0 weights) is subtracted from the saved t=0 column
#    before the final max-merge.  tanh is applied after the max (monotonic),
#    with scale 1/4096 folding out the fp8 scaling.
#  - l1..l4 (span means / boundary tokens) stay bf16: a small host-gathered
#    256-row token tile + bf16 mask matmul, f32 head as before.
import os
import sys

import numpy as np

for _p in ("/opt/trn_rl_repo", "/root/.axon_site/_ro/trn_rl_repo"):
    if _p not in sys.path and os.path.isdir(_p):
        sys.path.append(_p)

import ml_dtypes  # noqa: E402

B, L, E, P, V, FN, H2, LAB = 128, 128, 300, 50, 50000, 256, 100, 19
WIN = 3
FILTERS = (3, 4, 5)
NCORES = 8
BC = B // NCORES            # samples per core
LT = L + 2                  # padded token positions per sample
NTOK = BC * LT              # used token cols per chunk (2080)
CW = 2176                   # chunk width (17*128; cols 2080.. are zero pad)
POFF = 3 * CW               # pos region offset (6528)
NPOS = BC * L               # pos cols (2048)
WBIG = POFF + NPOS          # moving-tile width (8576)
ZCOL = NTOK                 # a guaranteed-zero column (chunk pad)
ECH = (128, 128, 44)        # E=300 split over partition chunks
NSEL = 6                    # l1, l2, l3a, l3b, l4a, l4b
SCALE = 64.0                # fp8 scaling for conv path (z domain = x4096)
BF16 = ml_dtypes.bfloat16
E4M3 = ml_dtypes.float8_e4m3

NPAIR = {3: 9, 4: 11, 5: 13}
PAIR_BASE = {3: 0, 4: 9, 5: 20}     # slot offset of each k's pairs in cwdr
NPAIR_ALL = 33
NCORP = 5                            # cor DR slots per (ki, mt)


def _members(k):
    return ([("we", s, ec) for s in range(k + 2) for ec in range(3)]
            + [("pos", j) for j in range(k)])


def _off0(m):
    # moving-data column offset at sample b=0
    if m[0] == "we":
        return m[2] * CW + m[1]
    return POFF + m[1]


def _off(m, b):
    if m[0] == "we":
        return m[2] * CW + 130 * b + m[1]
    return POFF + 128 * b + m[1]


def _pairs(k):
    ms = _members(k)
    prs = []
    for i in range(0, len(ms), 2):
        a, b = ms[i], ms[i + 1]
        if _off0(a) > _off0(b):
            a, b = b, a
        prs.append((a, b))
    assert len(prs) == NPAIR[k]
    return prs


def _cor_members():
    return [(m, ec) for m in range(WIN) for ec in range(3)]


def _cor_pairs():
    ms = _cor_members() + [None]     # pad to 10 with a zero member
    prs = []
    for i in range(0, len(ms), 2):
        a, b = ms[i], ms[i + 1]
        if b is not None and (a[1] * CW + a[0]) > (b[1] * CW + b[0]):
            a, b = b, a
        prs.append((a, b))
    assert len(prs) == NCORP
    return prs


def _prep_shared(emb, pos1, pos2, conv_w3, conv_b3, conv_w4, conv_b4,
                 conv_w5, conv_b5, W1, b1, W2, b2):
    """Host-side weight layout prep (replicated across cores)."""
    ws = {3: np.asarray(conv_w3, np.float32)[:, 0],
          4: np.asarray(conv_w4, np.float32)[:, 0],
          5: np.asarray(conv_w5, np.float32)[:, 0]}  # [FN, k, FD]

    # quantized tables (shared; per-core prep gathers from these)
    emb8 = (np.asarray(emb, np.float32) * SCALE).astype(E4M3)      # [V, E]
    emb16 = np.asarray(emb, np.float32).astype(BF16)               # [V, E]
    pos18 = (np.asarray(pos1, np.float32) * SCALE).astype(E4M3)    # [259, P]
    pos28 = (np.asarray(pos2, np.float32) * SCALE).astype(E4M3)

    # cwdr [NPAIR_ALL, 128, 512] fp8: DoubleRow-packed conv weights.
    # slot cols: i*256 + f  (member i of the pair, filter f of FN=256)
    cwdr = np.zeros((NPAIR_ALL, 128, 512), np.float32)
    for k in FILTERS:
        w = ws[k]
        wwe = np.zeros((k + 2, E, FN), np.float32)
        for j in range(k):
            for m in range(WIN):
                wwe[j + m] += w[:, j, E * m:E * (m + 1)].T
        wpos = np.stack([w[:, j, WIN * E:].T for j in range(k)])  # [k, 2P, FN]
        for pi, pr in enumerate(_pairs(k)):
            slot = PAIR_BASE[k] + pi
            for i, mem in enumerate(pr):
                if mem[0] == "we":
                    _, s, ec = mem
                    n = ECH[ec]
                    cwdr[slot, :n, i * 256:(i + 1) * 256] = \
                        wwe[s, 128 * ec:128 * ec + n]
                else:
                    _, j = mem
                    cwdr[slot, :2 * P, i * 256:(i + 1) * 256] = wpos[j]
    cwdr8 = (cwdr * SCALE).astype(E4M3)
    cwdr_flat = np.ascontiguousarray(
        cwdr8.transpose(1, 0, 2).reshape(128, NPAIR_ALL * 512))

    # cwcor [6*NCORP, 128, 256] fp8: t=0 correction (tap-0) DR weights.
    # slot index (2*ki+mt)*NCORP + pair; cols i*128 + f (128 filters of mt).
    cwcor = np.zeros((6 * NCORP, 128, 256), np.float32)
    for ki, k in enumerate(FILTERS):
        w0 = ws[k][:, 0]  # [FN, FD] tap j=0
        for mt in range(2):
            for pi, pr in enumerate(_cor_pairs()):
                slot = (2 * ki + mt) * NCORP + pi
                for i, mem in enumerate(pr):
                    if mem is None:
                        continue
                    m, ec = mem
                    n = ECH[ec]
                    cwcor[slot, :n, i * 128:(i + 1) * 128] = \
                        w0[mt * 128:(mt + 1) * 128,
                           E * m + 128 * ec:E * m + 128 * ec + n].T
    cwcor8 = (cwcor * SCALE).astype(E4M3)
    cwcor_flat = np.ascontiguousarray(
        cwcor8.transpose(1, 0, 2).reshape(128, 6 * NCORP * 256))

    # W1T: [128, 600]: col seg*100+h, rows = sf feature chunk seg
    w1t = np.ascontiguousarray(
        np.asarray(W1, np.float32).T.reshape(6, 128, H2)
        .transpose(1, 0, 2).reshape(128, 6 * H2))

    # W2T segs: 18 = (piece, ec) over l1,l2,l3a,l3b,l4a,l4b + 1 for g.
    W2 = np.asarray(W2, np.float32)
    segs = []
    for p in range(NSEL):
        for ec in range(3):
            seg = np.zeros((128, LAB), np.float32)
            seg[:ECH[ec]] = W2[:, E * p + 128 * ec:E * p + 128 * ec + ECH[ec]].T
            segs.append(seg)
    gseg = np.zeros((128, LAB), np.float32)
    gseg[:H2] = W2[:, NSEL * E:].T
    segs.append(gseg)
    w2t = np.ascontiguousarray(
        np.stack(segs).transpose(1, 0, 2).reshape(128, 19 * LAB))

    cb = np.zeros((128, 6), np.float32)
    for ki, k in enumerate(FILTERS):
        bk = {3: conv_b3, 4: conv_b4, 5: conv_b5}[k]
        cb[:, 2 * ki] = np.asarray(bk, np.float32)[:128]
        cb[:, 2 * ki + 1] = np.asarray(bk, np.float32)[128:]
    b1p = np.zeros((128, 1), np.float32)
    b1p[:H2, 0] = np.asarray(b1, np.float32)
    b2p = np.asarray(b2, np.float32).reshape(LAB, 1)

    return dict(cwdr=cwdr_flat, cwcor=cwcor_flat, w1t=w1t, w2t=w2t,
                cb=cb, b1=b1p, b2=b2p), emb8, emb16, pos18, pos28


def _prep_core(c, inputs, e1s, e1e, e2s, e2e, p1, p2,
               emb8, emb16, pos18, pos28):
    """Host-side per-core gather prep."""
    sl = slice(c * BC, (c + 1) * BC)
    inp = np.asarray(inputs[sl], np.int64)
    tok = np.zeros((BC, LT), np.int64)
    tok[:, 1:1 + L] = inp
    tok_flat = tok.reshape(-1)                       # [2080]

    # bigf8 [128, WBIG]: 3 feature chunks of the token stream + pos lookups
    big = np.zeros((128, WBIG), E4M3)
    we8 = emb8[tok_flat]                             # [2080, E] fp8
    for ec in range(3):
        n = ECH[ec]
        big[:n, ec * CW:ec * CW + NTOK] = \
            we8[:, 128 * ec:128 * ec + n].T
    p1f = np.asarray(p1[sl], np.int64).reshape(-1)   # [2048]
    p2f = np.asarray(p2[sl], np.int64).reshape(-1)
    big[:P, POFF:POFF + NPOS] = pos18[p1f].T
    big[P:2 * P, POFF:POFF + NPOS] = pos28[p2f].T

    # l-path: 16 selected rows per sample (spans + boundary tokens)
    sel_ids = np.zeros(2 * 128, np.int64)
    mask = np.zeros((2 * 128, BC * NSEL), np.float32)
    for lb in range(BC):
        b = c * BC + lb
        s1, t1 = int(e1s[b]), int(e1e[b])
        s2, t2 = int(e2s[b]), int(e2e[b])
        r = lb * 16
        ent = []
        cnt1 = t1 - s1 + 1
        for q in range(s1 + 1, t1 + 2):
            ent.append((q, 0, 1.0 / cnt1))
        cnt2 = t2 - s2 + 1
        for q in range(s2 + 1, t2 + 2):
            ent.append((q, 1, 1.0 / cnt2))
        ent += [(s1, 2, 1.0), (t1 + 2, 3, 1.0), (s2, 4, 1.0), (t2 + 2, 5, 1.0)]
        for j, (q, selk, val) in enumerate(ent):
            sel_ids[r + j] = tok[lb, q]
            mask[r + j, lb * NSEL + selk] = val
    seltok = np.ascontiguousarray(
        emb16[sel_ids].reshape(2, 128, E).transpose(1, 0, 2).reshape(128, 2 * E))
    selmask = np.ascontiguousarray(
        mask.astype(BF16).reshape(2, 128, BC * NSEL)
        .transpose(1, 0, 2).reshape(128, 2 * BC * NSEL))

    return dict(bigf8=big, seltok=seltok, selmask=selmask)


def _build_nc():
    import concourse.bacc as bacc
    import concourse.tile as tile
    from concourse import mybir
    from concourse.ap import AP

    f32, bf16, fp8 = mybir.dt.float32, mybir.dt.bfloat16, mybir.dt.float8e4
    DR = mybir.MatmulPerfMode.DoubleRow

    nc = bacc.Bacc("TRN2", target_bir_lowering=False, debug=False,
                   num_devices=NCORES)

    # ---- DRAM I/O (all partition-major; direct DMA) ----
    bigf8_d = nc.dram_tensor("bigf8", [128, WBIG], fp8, kind="ExternalInput")
    cwdr_d = nc.dram_tensor("cwdr", [128, NPAIR_ALL * 512], fp8,
                            kind="ExternalInput")
    cwcor_d = nc.dram_tensor("cwcor", [128, 6 * NCORP * 256], fp8,
                             kind="ExternalInput")
    seltok_d = nc.dram_tensor("seltok", [128, 2 * E], bf16, kind="ExternalInput")
    selmask_d = nc.dram_tensor("selmask", [128, 2 * BC * NSEL], bf16,
                               kind="ExternalInput")
    w1t_d = nc.dram_tensor("w1t", [128, 6 * H2], f32, kind="ExternalInput")
    w2t_d = nc.dram_tensor("w2t", [128, 19 * LAB], f32, kind="ExternalInput")
    cb_d = nc.dram_tensor("cb", [128, 6], f32, kind="ExternalInput")
    b1_d = nc.dram_tensor("b1", [128, 1], f32, kind="ExternalInput")
    b2_d = nc.dram_tensor("b2", [LAB, 1], f32, kind="ExternalInput")
    y_d = nc.dram_tensor("y", [LAB, BC], f32, kind="ExternalOutput")

    with tile.TileContext(nc) as tc:
        with tc.tile_pool(name="persist", bufs=1) as pp, \
             tc.tile_pool(name="l_ps", bufs=1, space="PSUM") as l_ps, \
             tc.tile_pool(name="z_ps", bufs=4, space="PSUM") as z_ps, \
             tc.tile_pool(name="aux_ps", bufs=1, space="PSUM") as aux_ps:

            # ---- persistent SBUF ----
            bigf8 = pp.tile([128, WBIG], fp8)
            cwdr_sb = pp.tile([128, NPAIR_ALL * 512], fp8)
            cwcor_sb = pp.tile([128, 6 * NCORP * 256], fp8)
            seltok_sb = pp.tile([128, 2 * E], bf16)
            selmask_sb = pp.tile([128, 2 * BC * NSEL], bf16)
            w1t_sb = pp.tile([128, 6 * H2], f32)
            w2t_sb = pp.tile([128, 19 * LAB], f32)
            cb_sb = pp.tile([128, 6], f32)
            b1_sb = pp.tile([128, 1], f32)
            b2_sb = pp.tile([LAB, 1], f32)
            lvec = [pp.tile([128, BC * NSEL], f32, tag=f"lvec{ec}",
                            name=f"lvec{ec}") for ec in range(3)]
            sf_sb = pp.tile([128, 6 * BC], f32)
            z0_sb = pp.tile([128, 6 * BC], f32)
            z0c_sb = pp.tile([128, 6 * BC], f32)
            cor_sb = pp.tile([128, 6 * BC], f32)
            sft = [pp.tile([128, BC], f32, tag=f"sft{i}", name=f"sft{i}")
                   for i in range(6)]
            g_sb = pp.tile([128, BC], f32)
            y_sb = pp.tile([LAB, BC], f32)

            # ---- DMA loads, critical-path order ----
            # conv needs bigf8 + cwdr(k) per k; cor/cwcor + head weights trail
            nc.sync.dma_start(bigf8[:], bigf8_d[:])
            cwk = {}
            for k in FILTERS:
                lo = PAIR_BASE[k] * 512
                hi = (PAIR_BASE[k] + NPAIR[k]) * 512
                cwk[k] = nc.sync.dma_start(cwdr_sb[:, lo:hi], cwdr_d[:, lo:hi])
            nc.sync.dma_start(seltok_sb[:], seltok_d[:])
            nc.sync.dma_start(selmask_sb[:], selmask_d[:])
            nc.sync.dma_start(cwcor_sb[:], cwcor_d[:])
            nc.sync.dma_start(w1t_sb[:], w1t_d[:])
            nc.sync.dma_start(w2t_sb[:], w2t_d[:])
            nc.sync.dma_start(cb_sb[:], cb_d[:])
            nc.sync.dma_start(b1_sb[:], b1_d[:])
            nc.sync.dma_start(b2_sb[:], b2_d[:])

            big_h = bigf8[:].tensor
            big_pstride = bigf8[:].ap[0][0]
            cw_h = cwdr_sb[:].tensor
            cw_pstride = cwdr_sb[:].ap[0][0]
            cc_h = cwcor_sb[:].tensor
            cc_pstride = cwcor_sb[:].ap[0][0]

            def big_ap(offa, offb, n, colstride=1):
                return AP(tensor=big_h, offset=offa,
                          ap=[[big_pstride, 128], [offb - offa, 2],
                              [colstride, n]])

            def cw_ap(slot, mt):
                off = slot * 512 + mt * 128
                return AP(tensor=cw_h, offset=off,
                          ap=[[cw_pstride, 128], [256, 2], [1, 128]])

            def cc_ap(slot):
                off = slot * 256
                return AP(tensor=cc_h, offset=off,
                          ap=[[cc_pstride, 128], [128, 2], [1, 128]])

            # ---- conv: fp8 DoubleRow chains ----
            for ki, k in enumerate(FILTERS):
                prs = _pairs(k)
                nz = L - k + 1
                # t0 chain: pairs containing a pos member; we-members read
                # the zero-pad column so t=0 gets pos-only contributions
                t0ch = []
                for pi, (a, b) in enumerate(prs):
                    if a[0] == "pos" or b[0] == "pos":
                        t0ch.append((pi, a, b))
                for mt in range(2):
                    blk = 2 * ki + mt
                    for grp in range(BC // 4):
                        zp = z_ps.tile([128, 4 * nz], f32, tag="zp")
                        zp3 = zp[:].rearrange("p (b t) -> p b t", b=4)
                        nmm = 4 * (len(t0ch) + len(prs))
                        n = 0
                        for bl in range(4):
                            bs = 4 * grp + bl
                            for pi, a, bm in t0ch:
                                oa = ZCOL if a[0] == "we" else _off(a, bs)
                                ob = ZCOL if bm[0] == "we" else _off(bm, bs)
                                if ob == oa:
                                    ob = oa + 1
                                nc.tensor.matmul(
                                    zp3[:, bl:bl + 1, 0:1],
                                    cw_ap(PAIR_BASE[k] + pi, mt),
                                    big_ap(oa, ob, 1),
                                    start=(n == 0), stop=(n == nmm - 1),
                                    perf_mode=DR)
                                n += 1
                            for pi, (a, bm) in enumerate(prs):
                                oa, ob = _off(a, bs) + 1, _off(bm, bs) + 1
                                nc.tensor.matmul(
                                    zp3[:, bl:bl + 1, 1:nz],
                                    cw_ap(PAIR_BASE[k] + pi, mt),
                                    big_ap(oa, ob, nz - 1),
                                    start=(n == 0), stop=(n == nmm - 1),
                                    perf_mode=DR)
                                n += 1
                        c0 = blk * BC + 4 * grp
                        nc.vector.tensor_copy(
                            z0_sb[:, c0:c0 + 4].rearrange(
                                "p (b o) -> p b o", o=1),
                            zp3[:, :, 0:1])
                        nc.vector.reduce_max(
                            sf_sb[:, c0:c0 + 4], zp3[:, :, 1:nz],
                            axis=mybir.AxisListType.X)

            # ---- l-path: bf16 span/boundary features ----
            lps = [l_ps.tile([128, BC * NSEL], f32, tag=f"lps{ec}",
                             name=f"lps{ec}") for ec in range(3)]
            for t in range(2):
                for ec in range(3):
                    nc.tensor.matmul(
                        lps[ec][:ECH[ec], :],
                        seltok_sb[:, t * E + 128 * ec:t * E + 128 * ec + ECH[ec]],
                        selmask_sb[:, t * BC * NSEL:(t + 1) * BC * NSEL],
                        start=(t == 0), stop=(t == 1))
            for ec in range(3):
                nc.vector.tensor_copy(lvec[ec][:ECH[ec], :],
                                      lps[ec][:ECH[ec], :])

            # ---- t=0 correction: c_k via fp8 DR, then merge into sf ----
            corp = aux_ps.tile([128, 6 * BC], f32, tag="aux")
            cprs = _cor_pairs()
            for ki in range(3):
                for mt in range(2):
                    blk = 2 * ki + mt
                    for pi, (a, bm) in enumerate(cprs):
                        oa = a[1] * CW + a[0]
                        ob = oa + 1 if bm is None else bm[1] * CW + bm[0]
                        nc.tensor.matmul(
                            corp[:, blk * BC:(blk + 1) * BC].rearrange(
                                "p (o b) -> p o b", o=1),
                            cc_ap(blk * NCORP + pi),
                            big_ap(oa, ob, BC, colstride=130),
                            start=(pi == 0), stop=(pi == NCORP - 1),
                            perf_mode=DR)
            nc.vector.tensor_copy(cor_sb[:], corp[:])
            nc.vector.tensor_sub(z0c_sb[:], z0_sb[:], cor_sb[:])
            nc.vector.tensor_max(sf_sb[:], sf_sb[:], z0c_sb[:])

            # ---- head: tanh (scale folds fp8 domain), g, y ----
            gp = aux_ps.tile([128, BC], f32, tag="aux")
            for i in range(6):
                nc.scalar.activation(sft[i][:], sf_sb[:, i * BC:(i + 1) * BC],
                                     mybir.ActivationFunctionType.Tanh,
                                     bias=cb_sb[:, i:i + 1],
                                     scale=1.0 / (SCALE * SCALE))
                nc.tensor.matmul(gp[:H2, :], w1t_sb[:, i * H2:(i + 1) * H2],
                                 sft[i][:], start=(i == 0), stop=(i == 5))
            nc.scalar.activation(g_sb[:H2, :], gp[:H2, :],
                                 mybir.ActivationFunctionType.Tanh,
                                 bias=b1_sb[:H2, :1])

            yp = aux_ps.tile([LAB, BC], f32, tag="aux")
            yp3 = yp[:].rearrange("p (o b) -> p o b", o=1)
            n = 0
            for p in range(NSEL):
                for ec in range(3):
                    nc.tensor.matmul(
                        yp3[:, :, :],
                        w2t_sb[:ECH[ec], (3 * p + ec) * LAB:
                               (3 * p + ec + 1) * LAB],
                        lvec[ec][:ECH[ec], :].rearrange(
                            "p (b s) -> p s b", s=NSEL)[:, p:p + 1, :],
                        start=(n == 0), stop=False)
                    n += 1
            nc.tensor.matmul(yp[:], w2t_sb[:H2, 18 * LAB:19 * LAB],
                             g_sb[:H2, :], start=False, stop=True)
            nc.scalar.activation(y_sb[:], yp[:],
                                 mybir.ActivationFunctionType.Identity,
                                 bias=b2_sb[:, :1])
            nc.sync.dma_start(y_d[:], y_sb[:])

    nc.compile()
    return nc


_NC = None
_LAST = None


def kernel(inputs, e1s, e1e, e2s, e2e, p1, p2, emb, pos1, pos2,
           conv_w3, conv_b3, conv_w4, conv_b4, conv_w5, conv_b5,
           W1, b1, W2, b2):
    global _NC
    from concourse.bass_utils import run_bass_kernel_spmd

    shared, emb8, emb16, pos18, pos28 = _prep_shared(
        emb, pos1, pos2, conv_w3, conv_b3, conv_w4, conv_b4,
        conv_w5, conv_b5, W1, b1, W2, b2)
    in_maps = []
    for c in range(NCORES):
        m = dict(shared)
        m.update(_prep_core(c, inputs, e1s, e1e, e2s, e2e, p1, p2,
                            emb8, emb16, pos18, pos28))
        in_maps.append(m)

    if _NC is None:
        _NC = _build_nc()

    trace = bool(int(os.environ.get("ATTCNN_TRACE", "0")))
    res = run_bass_kernel_spmd(_NC, in_maps, core_ids=list(range(NCORES)),
                               trace=trace)
    global _LAST
    _LAST = res
    y = np.zeros((B, LAB), np.float32)
    for c in range(NCORES):
        y[c * BC:(c + 1) * BC] = res.results[c]["y"].T
    return y


# revision 18
# speedup vs baseline: 1.1677x; 1.1677x over previous
# Trainium2 Bass kernel for nn_ATTCNN: embedding + window-CNN (k=3,4,5) +
# span-pool + MLP head. Data-parallel over 8 NeuronCores (16 samples each).
#
# Design (v3, fp8 DoubleRow + slab streaming):
#  - All input-dependent GATHERS run host-side (same class of prep as the
#    baseline's host-built one-hot matrices): the embedding window matrix is
#    shipped pre-transposed (feature-major) as fp8 e4m3 scaled by 64, in
#    sample-group-major slabs so conv compute streams behind 4 small DMAs.
#    All learned-weight arithmetic (conv, span means, MLP) runs on-device.
#  - Conv runs as fp8e4 DoubleRow matmuls: each matmul contracts 2 k-tiles
#    (pairs of (shift, feature-chunk) / pos-tap slices of the moving tile)
#    at 0.5 cycles/column.  The third feature chunk (44 rows of E=300) is
#    host-packed two shifts deep (rows 0..43 shift 2i, rows 44..87 shift
#    2i+1) so fewer k-tiles are needed.
#  - The reference zeroes WF[:, 0]: main chains cover t=1.., small pos-only
#    matmuls seed t=0, and the tap-0 correction c_k (fp8 DR vs saved t=0
#    column) is merged per conv-block: sf = max(reduce(z[1:]), z[0]-c).
#    tanh runs after the max (monotonic) with scale 1/4096 folding the fp8
#    scaling; the W1 accumulation is pipelined per block into the conv.
#  - l1..l4 (span means / boundary tokens) stay bf16: a small host-gathered
#    256-row token tile + bf16 mask matmul, f32 head.
import os
import sys

import numpy as np

for _p in ("/opt/trn_rl_repo", "/root/.axon_site/_ro/trn_rl_repo"):
    if _p not in sys.path and os.path.isdir(_p):
        sys.path.append(_p)

import ml_dtypes  # noqa: E402

B, L, E, P, V, FN, H2, LAB = 128, 128, 300, 50, 50000, 256, 100, 19
WIN = 3
FILTERS = (3, 4, 5)
NCORES = 8
BC = B // NCORES            # samples per core
LT = L + 2                  # padded token positions per sample
NSEL = 6                    # l1, l2, l3a, l3b, l4a, l4b
SCALE = 64.0                # fp8 scaling (z domain = x4096)
BF16 = ml_dtypes.bfloat16
E4M3 = ml_dtypes.float8_e4m3

# moving-tile layout: 4 slabs (4 samples each); per-slab regions.
# 8 zero cols lead each slab (t=0 DoubleRow reads for mixed we/pos pairs).
RZ, RCH0, RCH1, RPACK, RPOS = 0, 8, 528, 1048, 1568
SLABW = 2080
WBIG = 4 * SLABW + 8        # +8 tail pad for zero-member D=+1 overreads

NUNIT = {3: 8, 4: 10, 5: 12}
UNIT_BASE = {3: 0, 4: 8, 5: 18}
NSLOT = 30                  # conv DR weight slots
NCORP = 5                   # cor DR slots per (ki, mt)


def _members(k):
    npack = (k + 2 + 1) // 2
    return ([("ch", 0, s) for s in range(k + 2)]
            + [("ch", 1, s) for s in range(k + 2)]
            + [("pack", i) for i in range(npack)]
            + [("pos", j) for j in range(k)])


def _off0(m):
    if m is None:
        return 1 << 30
    if m[0] == "ch":
        return RCH0 + m[1] * 520 + m[2]
    if m[0] == "pack":
        return RPACK + 2 * m[1]
    return RPOS + m[1]


def _off(m, b):
    slab, bl = b // 4, b % 4
    base = slab * SLABW
    if m[0] == "ch":
        return base + RCH0 + m[1] * 520 + 130 * bl + m[2]
    if m[0] == "pack":
        return base + RPACK + 130 * bl + 2 * m[1]
    return base + RPOS + 128 * bl + m[1]


def _units(k):
    ms = _members(k)
    if len(ms) % 2:
        ms = ms + [None]
    prs = []
    for i in range(0, len(ms), 2):
        a, b = ms[i], ms[i + 1]
        if _off0(a) > _off0(b):
            a, b = b, a
        prs.append((a, b))
    assert len(prs) == NUNIT[k]
    return prs


def _cor_members():
    # (window m, feature chunk): ch0/ch1 full, chunk 2 lives in pack rows 0..43
    return [(m, ec) for m in range(WIN) for ec in range(3)]


def _cor_off0(mem):
    m, ec = mem
    return (RPACK if ec == 2 else RCH0 + ec * 520) + m


def _cor_units():
    ms = _cor_members() + [None]
    prs = []
    for i in range(0, len(ms), 2):
        a, b = ms[i], ms[i + 1]
        if b is not None and _cor_off0(a) > _cor_off0(b):
            a, b = b, a
        prs.append((a, b))
    assert len(prs) == NCORP
    return prs


def _prep_shared(emb, pos1, pos2, conv_w3, conv_b3, conv_w4, conv_b4,
                 conv_w5, conv_b5, W1, b1, W2, b2):
    """Host-side weight layout prep (replicated across cores)."""
    ws = {3: np.asarray(conv_w3, np.float32)[:, 0],
          4: np.asarray(conv_w4, np.float32)[:, 0],
          5: np.asarray(conv_w5, np.float32)[:, 0]}  # [FN, k, FD]

    emb8 = (np.asarray(emb, np.float32) * SCALE).astype(E4M3)      # [V, E]
    emb16 = np.asarray(emb, np.float32).astype(BF16)               # [V, E]
    pos18 = (np.asarray(pos1, np.float32) * SCALE).astype(E4M3)    # [259, P]
    pos28 = (np.asarray(pos2, np.float32) * SCALE).astype(E4M3)

    def member_rows(mem, wwe, wpos, k):
        # -> [128, FN] f32 weight rows for one pair member
        seg = np.zeros((128, FN), np.float32)
        if mem is None:
            return seg
        if mem[0] == "ch":
            _, ec, s = mem
            seg[:128] = wwe[s, 128 * ec:128 * (ec + 1)]
        elif mem[0] == "pack":
            i = mem[1]
            seg[:E - 256] = wwe[2 * i, 256:E]
            if 2 * i + 1 <= k + 1:
                seg[44:44 + E - 256] = wwe[2 * i + 1, 256:E]
        else:
            seg[:2 * P] = wpos[mem[1]]
        return seg

    cwdr = np.zeros((NSLOT, 128, 512), np.float32)
    for k in FILTERS:
        w = ws[k]
        wwe = np.zeros((k + 2, E, FN), np.float32)
        for j in range(k):
            for m in range(WIN):
                wwe[j + m] += w[:, j, E * m:E * (m + 1)].T
        wpos = np.stack([w[:, j, WIN * E:].T for j in range(k)])  # [k, 2P, FN]
        for ui, (a, b) in enumerate(_units(k)):
            slot = UNIT_BASE[k] + ui
            cwdr[slot, :, 0:256] = member_rows(a, wwe, wpos, k)
            cwdr[slot, :, 256:512] = member_rows(b, wwe, wpos, k)
    cwdr8 = (cwdr * SCALE).astype(E4M3)
    cwdr_flat = np.ascontiguousarray(
        cwdr8.transpose(1, 0, 2).reshape(128, NSLOT * 512))

    # cwcor [6*NCORP, 128, 256] fp8: tap-0 correction DR weights.
    cwcor = np.zeros((6 * NCORP, 128, 256), np.float32)
    for ki, k in enumerate(FILTERS):
        w0 = ws[k][:, 0]  # [FN, FD] tap j=0
        for mt in range(2):
            for pi, pr in enumerate(_cor_units()):
                slot = (2 * ki + mt) * NCORP + pi
                for i, mem in enumerate(pr):
                    if mem is None:
                        continue
                    m, ec = mem
                    n = 128 if ec < 2 else E - 256
                    cwcor[slot, :n, i * 128:(i + 1) * 128] = \
                        w0[mt * 128:(mt + 1) * 128,
                           E * m + 128 * ec:E * m + 128 * ec + n].T
    cwcor8 = (cwcor * SCALE).astype(E4M3)
    cwcor_flat = np.ascontiguousarray(
        cwcor8.transpose(1, 0, 2).reshape(128, 6 * NCORP * 256))

    w1t = np.ascontiguousarray(
        np.asarray(W1, np.float32).T.reshape(6, 128, H2)
        .transpose(1, 0, 2).reshape(128, 6 * H2))

    W2 = np.asarray(W2, np.float32)
    ech = (128, 128, 44)
    segs = []
    for p in range(NSEL):
        for ec in range(3):
            seg = np.zeros((128, LAB), np.float32)
            seg[:ech[ec]] = W2[:, E * p + 128 * ec:E * p + 128 * ec + ech[ec]].T
            segs.append(seg)
    gseg = np.zeros((128, LAB), np.float32)
    gseg[:H2] = W2[:, NSEL * E:].T
    segs.append(gseg)
    w2t = np.ascontiguousarray(
        np.stack(segs).transpose(1, 0, 2).reshape(128, 19 * LAB))

    cb = np.zeros((128, 6), np.float32)
    for ki, k in enumerate(FILTERS):
        bk = {3: conv_b3, 4: conv_b4, 5: conv_b5}[k]
        cb[:, 2 * ki] = np.asarray(bk, np.float32)[:128]
        cb[:, 2 * ki + 1] = np.asarray(bk, np.float32)[128:]
    b1p = np.zeros((128, 1), np.float32)
    b1p[:H2, 0] = np.asarray(b1, np.float32)
    b2p = np.asarray(b2, np.float32).reshape(LAB, 1)

    return dict(cwdr=cwdr_flat, cwcor=cwcor_flat, w1t=w1t, w2t=w2t,
                cb=cb, b1=b1p, b2=b2p), emb8, emb16, pos18, pos28


def _prep_core(c, inputs, e1s, e1e, e2s, e2e, p1, p2,
               emb8, emb16, pos18, pos28):
    """Host-side per-core gather prep."""
    sl = slice(c * BC, (c + 1) * BC)
    inp = np.asarray(inputs[sl], np.int64)
    tok = np.zeros((BC, LT), np.int64)
    tok[:, 1:1 + L] = inp

    p1f = np.asarray(p1[sl], np.int64)
    p2f = np.asarray(p2[sl], np.int64)

    big = np.zeros((128, WBIG), E4M3)
    for slab in range(4):
        base = slab * SLABW
        stok = tok[4 * slab:4 * slab + 4].reshape(-1)      # [520]
        we8 = emb8[stok]                                   # [520, E]
        big[:, base + RCH0:base + RCH0 + 520] = we8[:, 0:128].T
        big[:, base + RCH1:base + RCH1 + 520] = we8[:, 128:256].T
        big[:E - 256, base + RPACK:base + RPACK + 520] = we8[:, 256:E].T
        big[44:44 + E - 256, base + RPACK:base + RPACK + 519] = \
            we8[1:, 256:E].T
        sp1 = pos18[p1f[4 * slab:4 * slab + 4].reshape(-1)]  # [512, P]
        sp2 = pos28[p2f[4 * slab:4 * slab + 4].reshape(-1)]
        big[:P, base + RPOS:base + RPOS + 512] = sp1.T
        big[P:2 * P, base + RPOS:base + RPOS + 512] = sp2.T
        big[:, base + RZ:base + RZ + 8] = 0.0

    # l-path: 16 selected rows per sample (spans + boundary tokens)
    sel_ids = np.zeros(2 * 128, np.int64)
    mask = np.zeros((2 * 128, BC * NSEL), np.float32)
    for lb in range(BC):
        b = c * BC + lb
        s1, t1 = int(e1s[b]), int(e1e[b])
        s2, t2 = int(e2s[b]), int(e2e[b])
        r = lb * 16
        ent = []
        cnt1 = t1 - s1 + 1
        for q in range(s1 + 1, t1 + 2):
            ent.append((q, 0, 1.0 / cnt1))
        cnt2 = t2 - s2 + 1
        for q in range(s2 + 1, t2 + 2):
            ent.append((q, 1, 1.0 / cnt2))
        ent += [(s1, 2, 1.0), (t1 + 2, 3, 1.0), (s2, 4, 1.0), (t2 + 2, 5, 1.0)]
        for j, (q, selk, val) in enumerate(ent):
            sel_ids[r + j] = tok[lb, q]
            mask[r + j, lb * NSEL + selk] = val
    seltok = np.ascontiguousarray(
        emb16[sel_ids].reshape(2, 128, E).transpose(1, 0, 2).reshape(128, 2 * E))
    selmask = np.ascontiguousarray(
        mask.astype(BF16).reshape(2, 128, BC * NSEL)
        .transpose(1, 0, 2).reshape(128, 2 * BC * NSEL))

    return dict(bigf8=big, seltok=seltok, selmask=selmask)


def _build_nc():
    import concourse.bacc as bacc
    import concourse.tile as tile
    from concourse import mybir
    from concourse.ap import AP

    f32, bf16, fp8 = mybir.dt.float32, mybir.dt.bfloat16, mybir.dt.float8e4
    DR = mybir.MatmulPerfMode.DoubleRow
    ech = (128, 128, 44)

    nc = bacc.Bacc("TRN2", target_bir_lowering=False, debug=False,
                   num_devices=NCORES)

    bigf8_d = nc.dram_tensor("bigf8", [128, WBIG], fp8, kind="ExternalInput")
    cwdr_d = nc.dram_tensor("cwdr", [128, NSLOT * 512], fp8,
                            kind="ExternalInput")
    cwcor_d = nc.dram_tensor("cwcor", [128, 6 * NCORP * 256], fp8,
                             kind="ExternalInput")
    seltok_d = nc.dram_tensor("seltok", [128, 2 * E], bf16, kind="ExternalInput")
    selmask_d = nc.dram_tensor("selmask", [128, 2 * BC * NSEL], bf16,
                               kind="ExternalInput")
    w1t_d = nc.dram_tensor("w1t", [128, 6 * H2], f32, kind="ExternalInput")
    w2t_d = nc.dram_tensor("w2t", [128, 19 * LAB], f32, kind="ExternalInput")
    cb_d = nc.dram_tensor("cb", [128, 6], f32, kind="ExternalInput")
    b1_d = nc.dram_tensor("b1", [128, 1], f32, kind="ExternalInput")
    b2_d = nc.dram_tensor("b2", [LAB, 1], f32, kind="ExternalInput")
    y_d = nc.dram_tensor("y", [LAB, BC], f32, kind="ExternalOutput")

    with tile.TileContext(nc) as tc:
        with tc.tile_pool(name="persist", bufs=1) as pp, \
             tc.tile_pool(name="l_ps", bufs=1, space="PSUM") as l_ps, \
             tc.tile_pool(name="z_ps", bufs=4, space="PSUM") as z_ps, \
             tc.tile_pool(name="aux_ps", bufs=1, space="PSUM") as aux_ps:

            bigf8 = pp.tile([128, WBIG], fp8)
            cwdr_sb = pp.tile([128, NSLOT * 512], fp8)
            cwcor_sb = pp.tile([128, 6 * NCORP * 256], fp8)
            seltok_sb = pp.tile([128, 2 * E], bf16)
            selmask_sb = pp.tile([128, 2 * BC * NSEL], bf16)
            w1t_sb = pp.tile([128, 6 * H2], f32)
            w2t_sb = pp.tile([128, 19 * LAB], f32)
            cb_sb = pp.tile([128, 6], f32)
            b1_sb = pp.tile([128, 1], f32)
            b2_sb = pp.tile([LAB, 1], f32)
            lvec = [pp.tile([128, BC * NSEL], f32, tag=f"lvec{ec}",
                            name=f"lvec{ec}") for ec in range(3)]
            sf_sb = pp.tile([128, 6 * BC], f32)
            z0_sb = pp.tile([128, 6 * BC], f32)
            z0c_sb = pp.tile([128, 6 * BC], f32)
            cor_sb = pp.tile([128, 6 * BC], f32)
            sft = [pp.tile([128, BC], f32, tag=f"sft{i}", name=f"sft{i}")
                   for i in range(6)]
            g_sb = pp.tile([128, BC], f32)
            y_sb = pp.tile([LAB, BC], f32)

            # ---- DMA loads, critical-path order ----
            for slab in range(4):
                hi = WBIG if slab == 3 else (slab + 1) * SLABW
                nc.sync.dma_start(
                    bigf8[:, slab * SLABW:hi],
                    bigf8_d[:, slab * SLABW:hi])
                if slab == 0:
                    lo, hi = UNIT_BASE[3] * 512, (UNIT_BASE[3] + NUNIT[3]) * 512
                    nc.sync.dma_start(cwdr_sb[:, lo:hi], cwdr_d[:, lo:hi])
            lo, hi = UNIT_BASE[4] * 512, (UNIT_BASE[4] + NUNIT[4]) * 512
            nc.sync.dma_start(cwdr_sb[:, lo:hi], cwdr_d[:, lo:hi])
            nc.sync.dma_start(seltok_sb[:], seltok_d[:])
            nc.sync.dma_start(selmask_sb[:], selmask_d[:])
            nc.sync.dma_start(w1t_sb[:], w1t_d[:])
            nc.sync.dma_start(w2t_sb[:], w2t_d[:])
            nc.sync.dma_start(cb_sb[:], cb_d[:])
            nc.sync.dma_start(cwcor_sb[:], cwcor_d[:])
            lo, hi = UNIT_BASE[5] * 512, (UNIT_BASE[5] + NUNIT[5]) * 512
            nc.sync.dma_start(cwdr_sb[:, lo:hi], cwdr_d[:, lo:hi])
            nc.sync.dma_start(b1_sb[:], b1_d[:])
            nc.sync.dma_start(b2_sb[:], b2_d[:])

            big_h = bigf8[:].tensor
            big_ps = bigf8[:].ap[0][0]
            cw_h = cwdr_sb[:].tensor
            cw_ps = cwdr_sb[:].ap[0][0]
            cc_h = cwcor_sb[:].tensor
            cc_ps = cwcor_sb[:].ap[0][0]

            def big_ap(offa, offb, n, colstride=1):
                return AP(tensor=big_h, offset=offa,
                          ap=[[big_ps, 128], [offb - offa, 2], [colstride, n]])

            def big_ap1(off, n, colstride=1):
                return AP(tensor=big_h, offset=off,
                          ap=[[big_ps, 128], [colstride, n]])

            def cw_ap(slot, mt):
                return AP(tensor=cw_h, offset=slot * 512 + mt * 128,
                          ap=[[cw_ps, 128], [256, 2], [1, 128]])

            def cw_ap1(slot, i, mt):
                return AP(tensor=cw_h, offset=slot * 512 + i * 256 + mt * 128,
                          ap=[[cw_ps, 128], [1, 128]])

            def cc_ap(slot):
                return AP(tensor=cc_h, offset=slot * 256,
                          ap=[[cc_ps, 128], [128, 2], [1, 128]])

            def conv_block(ki, merge_now):
                k = FILTERS[ki]
                units = _units(k)
                nz = L - k + 1
                t0ch = []
                for ui, (a, b) in enumerate(units):
                    apos = a is not None and a[0] == "pos"
                    bpos = b is not None and b[0] == "pos"
                    if apos and bpos:
                        t0ch.append(("dr", ui))
                    elif apos and b is None:
                        t0ch.append(("drz", ui))
                    elif bpos:
                        t0ch.append(("single", ui))
                for mt in range(2):
                    blk = 2 * ki + mt
                    for grp in range(BC // 4):
                        zp = z_ps.tile([128, 4 * nz], f32, tag="zp")
                        zp3 = zp[:].rearrange("p (b t) -> p b t", b=4)
                        nmm = 4 * (len(t0ch) + len(units))
                        n = 0
                        for bl in range(4):
                            bs = 4 * grp + bl
                            for kind, ui in t0ch:
                                a, b = units[ui]
                                slot = UNIT_BASE[k] + ui
                                if kind == "single":
                                    # mixed (we, pos): we member reads the
                                    # slab's zero columns
                                    oa = (bs // 4) * SLABW + RZ
                                    ob = _off(b, bs)
                                else:
                                    oa = _off(a, bs)
                                    ob = oa + 2 if b is None else _off(b, bs)
                                nc.tensor.matmul(
                                    zp3[:, bl:bl + 1, 0:1],
                                    cw_ap(slot, mt),
                                    big_ap(oa, ob, 1),
                                    start=(n == 0), stop=(n == nmm - 1),
                                    perf_mode=DR)
                                n += 1
                            for ui, (a, b) in enumerate(units):
                                oa = _off(a, bs) + 1
                                ob = oa + 2 if b is None else _off(b, bs) + 1
                                nc.tensor.matmul(
                                    zp3[:, bl:bl + 1, 1:nz],
                                    cw_ap(UNIT_BASE[k] + ui, mt),
                                    big_ap(oa, ob, nz - 1),
                                    start=(n == 0), stop=(n == nmm - 1),
                                    perf_mode=DR)
                                n += 1
                        c0 = blk * BC + 4 * grp
                        nc.vector.tensor_copy(
                            z0_sb[:, c0:c0 + 4].rearrange(
                                "p (b o) -> p b o", o=1),
                            zp3[:, :, 0:1])
                        nc.vector.reduce_max(
                            sf_sb[:, c0:c0 + 4], zp3[:, :, 1:nz],
                            axis=mybir.AxisListType.X)
                    if merge_now:
                        merge_block(blk)

            def merge_block(blk):
                c0, c1 = blk * BC, (blk + 1) * BC
                nc.vector.tensor_sub(z0c_sb[:, c0:c1], z0_sb[:, c0:c1],
                                     cor_sb[:, c0:c1])
                nc.vector.tensor_max(sf_sb[:, c0:c1], sf_sb[:, c0:c1],
                                     z0c_sb[:, c0:c1])
                nc.scalar.activation(sft[blk][:], sf_sb[:, c0:c1],
                                     mybir.ActivationFunctionType.Tanh,
                                     bias=cb_sb[:, blk:blk + 1],
                                     scale=1.0 / (SCALE * SCALE))

            # k=3, k=4 conv blocks (merges deferred until cor is in)
            conv_block(0, merge_now=False)
            conv_block(1, merge_now=False)

            # ---- t=0 correction chains (PE caught up; cwcor arrived) ----
            corp = aux_ps.tile([128, 6 * BC], f32, tag="aux")
            cunits = _cor_units()
            n = 0
            ncm = 6 * 4 * NCORP
            for ki in range(3):
                for mt in range(2):
                    blk = 2 * ki + mt
                    for slab in range(4):
                        for pi, (a, b) in enumerate(cunits):
                            oa = slab * SLABW + _cor_off0(a)
                            ob = oa + 2 if b is None else \
                                slab * SLABW + _cor_off0(b)
                            nc.tensor.matmul(
                                corp[:, blk * BC + 4 * slab:
                                     blk * BC + 4 * slab + 4].rearrange(
                                    "p (o b) -> p o b", o=1),
                                cc_ap(blk * NCORP + pi),
                                big_ap(oa, ob, 4, colstride=130),
                                start=(n == 0), stop=(n == ncm - 1),
                                perf_mode=DR)
                            n += 1
            nc.vector.tensor_copy(cor_sb[:], corp[:])

            # ---- l-path: bf16 span/boundary features ----
            lps = [l_ps.tile([128, BC * NSEL], f32, tag=f"lps{ec}",
                             name=f"lps{ec}") for ec in range(3)]
            for t in range(2):
                for ec in range(3):
                    nc.tensor.matmul(
                        lps[ec][:ech[ec], :],
                        seltok_sb[:, t * E + 128 * ec:t * E + 128 * ec + ech[ec]],
                        selmask_sb[:, t * BC * NSEL:(t + 1) * BC * NSEL],
                        start=(t == 0), stop=(t == 1))
            for ec in range(3):
                nc.vector.tensor_copy(lvec[ec][:ech[ec], :],
                                      lps[ec][:ech[ec], :])

            # deferred merges for k=3/k=4 blocks, then k=5 with inline merges
            for blk in range(4):
                merge_block(blk)
            conv_block(2, merge_now=True)

            # ---- head tail: g chain, then y chain (aux bank reused) ----
            gp = aux_ps.tile([128, BC], f32, tag="aux")
            for i in range(6):
                nc.tensor.matmul(gp[:H2, :], w1t_sb[:, i * H2:(i + 1) * H2],
                                 sft[i][:], start=(i == 0), stop=(i == 5))
            nc.scalar.activation(g_sb[:H2, :], gp[:H2, :],
                                 mybir.ActivationFunctionType.Tanh,
                                 bias=b1_sb[:H2, :1])

            yp = aux_ps.tile([LAB, BC], f32, tag="aux")
            yp3 = yp[:].rearrange("p (o b) -> p o b", o=1)
            n = 0
            for p in range(NSEL):
                for ec in range(3):
                    nc.tensor.matmul(
                        yp3[:, :, :],
                        w2t_sb[:ech[ec], (3 * p + ec) * LAB:
                               (3 * p + ec + 1) * LAB],
                        lvec[ec][:ech[ec], :].rearrange(
                            "p (b s) -> p s b", s=NSEL)[:, p:p + 1, :],
                        start=(n == 0), stop=False)
                    n += 1
            nc.tensor.matmul(yp[:], w2t_sb[:H2, 18 * LAB:19 * LAB],
                             g_sb[:H2, :], start=False, stop=True)
            nc.scalar.activation(y_sb[:], yp[:],
                                 mybir.ActivationFunctionType.Identity,
                                 bias=b2_sb[:, :1])
            nc.sync.dma_start(y_d[:], y_sb[:])

    nc.compile()
    return nc


_NC = None
_LAST = None


def kernel(inputs, e1s, e1e, e2s, e2e, p1, p2, emb, pos1, pos2,
           conv_w3, conv_b3, conv_w4, conv_b4, conv_w5, conv_b5,
           W1, b1, W2, b2):
    global _NC
    from concourse.bass_utils import run_bass_kernel_spmd

    shared, emb8, emb16, pos18, pos28 = _prep_shared(
        emb, pos1, pos2, conv_w3, conv_b3, conv_w4, conv_b4,
        conv_w5, conv_b5, W1, b1, W2, b2)
    in_maps = []
    for c in range(NCORES):
        m = dict(shared)
        m.update(_prep_core(c, inputs, e1s, e1e, e2s, e2e, p1, p2,
                            emb8, emb16, pos18, pos28))
        in_maps.append(m)

    if _NC is None:
        _NC = _build_nc()

    trace = bool(int(os.environ.get("ATTCNN_TRACE", "0")))
    res = run_bass_kernel_spmd(_NC, in_maps, core_ids=list(range(NCORES)),
                               trace=trace)
    global _LAST
    _LAST = res
    y = np.zeros((B, LAB), np.float32)
    for c in range(NCORES):
        y[c * BC:(c + 1) * BC] = res.results[c]["y"].T
    return y
